# revision 4
# baseline (speedup 1.0000x reference)
"""CTC loss (keras ctc_batch_cost semantics) on 8 Trainium2 NeuronCores.

Strategy (data parallel, 32 samples/core):
  Prob-domain CTC forward with per-sample/per-block prescaling.  The time
  recursion alpha_t = (c_{t-1} + alpha_{t-1}) * p_t is computed row-by-row
  (row = extended-label state s) with the DVE tensor_tensor_scan instruction
  (op0=add, op1=mult), one scan per (row, 128-step time block).

  Layout: partitions = (sample_local b in 0..31) x (time block tau in 0..3),
  free dim = t within block.  Work is ordered by skewed diagonals
  d = s + 2*tau so every diagonal has uniform blank/label parity and all
  cross-row references stay in-partition; the only cross-partition value is
  the scan's initial carry at block boundaries, produced by a tiny PE
  shift-matrix matmul accumulated into PSUM (scan reads `initial` from PSUM).

  Host-side prep (numpy): label-indexed gather of emissions into the skewed
  layout, bf16 cast, per-sample per-block power-of-two-ish prescale chosen
  from a coarse float64 estimate (pure preconditioning - correctness never
  depends on it; exact log-scale corrections are folded into the final loss
  constant per sample).
"""

import numpy as np
import ml_dtypes

B, T, C, L = 256, 512, 128, 64
S = 2 * L + 1          # 129 extended states
BLANK = C - 1
EPS = 1e-7
W = 128                # time-block width
K = 4                  # number of time blocks (T = K*W)
ND = S + 2 * (K - 1)   # diagonals: d = s + 2*tau in [0, 134]
NODD = (ND + 1) // 2   # odd diagonals (label rows)
NCORES = 8
BC = B // NCORES       # 32 samples per core
P = 128                # partitions = BC * K

_PROG_CACHE = {}


def _build_program():
    import concourse.bass as bass
    import concourse.bacc as bacc
    import concourse.mybir as mybir
    import concourse.tile as tile

    f32 = mybir.dt.float32
    bf16 = mybir.dt.bfloat16
    ADD = mybir.AluOpType.add
    MULT = mybir.AluOpType.mult

    nc = bacc.Bacc("TRN2", target_bir_lowering=False, debug=False)

    p_dram = nc.dram_tensor("p_skew", [P, ND * W], bf16, kind="ExternalInput")
    mu_dram = nc.dram_tensor("mu", [P, ND], f32, kind="ExternalInput")
    sel_dram = nc.dram_tensor("sel", [P, ND], bf16, kind="ExternalInput")
    corr_dram = nc.dram_tensor("corr", [P, 1], f32, kind="ExternalInput")
    sh_dram = nc.dram_tensor("sh", [P, P], bf16, kind="ExternalInput")
    out_dram = nc.dram_tensor("loss_out", [P, 1], f32, kind="ExternalOutput")

    with tile.TileContext(nc) as tc:
        with (
            tc.tile_pool(name="stat", bufs=1) as stat,
            tc.tile_pool(name="psum", bufs=4, space="PSUM") as psum,
        ):
            p_sb = stat.tile([P, ND * W], bf16, tag="p_sb")
            abuf = stat.tile([P, ND * (W + 1)], bf16, tag="abuf")
            cbuf = stat.tile([P, NODD * (W + 1)], bf16, tag="cbuf")
            wbuf = stat.tile([P, NODD * W], bf16, tag="wbuf")
            zw = stat.tile([P, W], bf16, tag="zw")
            mu_sb = stat.tile([P, ND], f32, tag="mu_sb")
            sel_sb = stat.tile([P, ND], bf16, tag="sel_sb")
            corr_sb = stat.tile([P, 1], f32, tag="corr_sb")
            sh_sb = stat.tile([P, P], bf16, tag="sh_sb")
            rsel = stat.tile([P, ND], bf16, tag="rsel")
            r_col = stat.tile([P, 1], f32, tag="r_col")
            lnr = stat.tile([P, 1], f32, tag="lnr")
            eps_col = stat.tile([P, 1], f32, tag="eps_col")
            loss_sb = stat.tile([P, 1], f32, tag="loss_sb")

            # --- loads ---------------------------------------------------
            nc.sync.dma_start(out=sh_sb[:], in_=sh_dram[:])
            nc.sync.dma_start(out=mu_sb[:], in_=mu_dram[:])
            nc.sync.dma_start(out=sel_sb[:], in_=sel_dram[:])
            nc.sync.dma_start(out=corr_sb[:], in_=corr_dram[:])
            CH = 15  # diagonals per p_skew load chunk
            for i in range(0, ND, CH):
                wch = min(CH, ND - i)
                nc.sync.dma_start(
                    out=p_sb[:, i * W:(i + wch) * W],
                    in_=p_dram[:, i * W:(i + wch) * W],
                )

            # --- zero pads ----------------------------------------------
            a3 = abuf[:].rearrange("p (d c) -> p d c", c=W + 1)
            c3 = cbuf[:].rearrange("p (d c) -> p d c", c=W + 1)
            nc.vector.memset(a3[:, :, 0:1], 0.0)
            nc.vector.memset(c3[:, :, 0:1], 0.0)
            nc.vector.memset(zw[:], 0.0)
            nc.vector.memset(eps_col[:], 1e-35)

            def acol(d, i):
                return abuf[:, d * (W + 1) + i: d * (W + 1) + i + 1]

            def ccol(oi, i):
                return cbuf[:, oi * (W + 1) + i: oi * (W + 1) + i + 1]

            # --- wavefront ----------------------------------------------
            for d in range(ND):
                odd = (d % 2) == 1
                oi = (d - 1) // 2

                # boundary initial (PSUM) for d >= 2
                if d >= 2:
                    pt = psum.tile([P, 1], f32, tag="init")
                    rhs1 = acol(d - 2, W)
                    if odd:
                        rhs2 = ccol((d - 3) // 2, W)
                    else:
                        rhs2 = acol(d - 3, W) if d >= 3 else None
                    nc.tensor.matmul(
                        pt[:], sh_sb[:], rhs1, start=True, stop=rhs2 is None
                    )
                    if rhs2 is not None:
                        nc.tensor.matmul(
                            pt[:], sh_sb[:], rhs2, start=False, stop=True
                        )
                    initial = pt[:, 0:1]
                else:
                    initial = 1.0

                # c tile (odd d only; even d reads alpha(d-1) directly)
                if odd:
                    w_in = zw[:] if d == 1 else wbuf[:, ((d - 3) // 2) * W:((d - 3) // 2) * W + W]
                    nc.vector.tensor_add(
                        cbuf[:, oi * (W + 1) + 1: oi * (W + 1) + 1 + W],
                        abuf[:, (d - 1) * (W + 1) + 1: (d - 1) * (W + 1) + 1 + W],
                        w_in,
                    )
                    data0 = cbuf[:, oi * (W + 1): oi * (W + 1) + W]
                else:
                    if d == 0:
                        data0 = zw[:]
                    else:
                        data0 = abuf[:, (d - 1) * (W + 1): (d - 1) * (W + 1) + W]

                nc.vector.tensor_tensor_scan(
                    abuf[:, d * (W + 1) + 1: d * (W + 1) + 1 + W],
                    data0,
                    p_sb[:, d * W:(d + 1) * W],
                    initial,
                    op0=ADD,
                    op1=MULT,
                )

                # wtilde for label rows (consumed at d+2)
                if odd and d + 2 < ND:
                    nc.scalar.activation(
                        wbuf[:, oi * W: oi * W + W],
                        abuf[:, d * (W + 1) + 1: d * (W + 1) + 1 + W],
                        mybir.ActivationFunctionType.Copy,
                        scale=mu_sb[:, d:d + 1],
                    )

            # --- readout -------------------------------------------------
            lastcols = a3[:, :, W]  # [P, ND]
            nc.vector.tensor_mul(rsel[:], lastcols, sel_sb[:])
            nc.vector.tensor_reduce(
                r_col[:], rsel[:], axis=mybir.AxisListType.X, op=ADD
            )
            nc.scalar.activation(
                lnr[:], r_col[:], mybir.ActivationFunctionType.Ln,
                bias=eps_col[:, 0:1],
            )
            nc.vector.tensor_scalar(
                loss_sb[:], lnr[:], -1.0, corr_sb[:, 0:1], op0=MULT, op1=ADD
            )
            nc.sync.dma_start(out=out_dram[:], in_=loss_sb[:])

    nc.compile()
    return nc


def _host_prep(y_pred, labels, label_len):
    """Build per-core device inputs. Returns list of in_maps + nothing else."""
    ll = label_len[:, 0].astype(np.int64)
    pe = y_pred.astype(np.float64) + EPS
    cls = np.full((B, S), BLANK, np.int64)
    cls[:, 1::2] = labels

    # gathered emissions [B, S, T], valid-masked (s <= 2*ll)
    em = np.take_along_axis(pe.transpose(0, 2, 1), cls[:, :, None], axis=1)
    valid = np.arange(S)[None, :] <= (2 * ll[:, None])
    em = em * valid[:, :, None]

    # skip mask per row
    mt = np.zeros((B, S), np.float32)
    j = np.arange(1, L)
    mt[:, 2 * j + 1] = (labels[:, j] != labels[:, j - 1]).astype(np.float32)

    # --- coarse scale estimation (float64, preconditioning only) --------
    SUB = 4
    mt64 = mt.astype(np.float64)
    a = np.zeros((B, S))
    a[:, 0] = em[:, 0, 0]
    a[:, 1] = em[:, 1, 0]
    logsc = np.zeros(B)
    blk_log = np.zeros((B, K))
    emc = em.reshape(B, S, T // SUB, SUB)
    for tc_ in range(1, T // SUB):
        pstep = emc[:, :, tc_, :].prod(axis=2) ** (1.0 / SUB)
        for _ in range(SUB):
            s1 = np.concatenate([np.zeros((B, 1)), a[:, :-1]], 1)
            s2 = np.concatenate([np.zeros((B, 2)), a[:, :-2]], 1) * mt64
            a = (a + s1 + s2) * pstep
        m = np.maximum(a.max(1), 1e-300)
        logsc += np.log2(m)
        a = a / m[:, None]
        blk_log[:, min(tc_ * SUB // W, K - 1)] = logsc
    deltas = np.diff(np.concatenate([np.zeros((B, 1)), blk_log], 1), axis=1)
    g = np.clip(-deltas / W, -30, 30)
    scale = (2.0 ** g).astype(np.float32)                     # [B, K]
    corr_all = (W * np.log(scale.astype(np.float64))).sum(1)  # [B] nats

    # scaled bf16 emissions
    emsc = np.zeros((B, S, T), np.float32)
    for tau in range(K):
        emsc[:, :, tau * W:(tau + 1) * W] = (
            em[:, :, tau * W:(tau + 1) * W] * scale[:, None, tau, None]
        )
    emsc = emsc.astype(ml_dtypes.bfloat16)

    # shift matrix (shared by all cores): out[p] = in[p-1] if p%4 != 0
    sh = np.zeros((P, P), np.float32)
    for p in range(P):
        if p % K != 0:
            sh[p - 1, p] = 1.0
    sh = sh.astype(ml_dtypes.bfloat16)

    in_maps = []
    for c in range(NCORES):
        bs = slice(c * BC, (c + 1) * BC)
        llc = ll[bs]
        # p_skew [BC, K, ND, W] -> [P, ND*W]
        psk = np.zeros((BC, K, ND, W), ml_dtypes.bfloat16)
        for tau in range(K):
            for d in range(2 * tau, 2 * tau + S):
                s = d - 2 * tau
                psk[:, tau, d, :] = emsc[bs, s, tau * W:(tau + 1) * W]
        p_skew = psk.reshape(BC * K, ND * W)

        # mu [P, ND]: mu[(b,tau), d] = mt[b, d-2tau+2] for odd d
        muc = np.zeros((BC, K, ND), np.float32)
        for tau in range(K):
            for d in range(1, ND, 2):
                s2 = d - 2 * tau + 2
                if 1 <= s2 < S and s2 % 2 == 1:
                    muc[:, tau, d] = mt[bs, s2]
        mu = muc.reshape(P, ND)

        # sel [P, ND]: pick alpha-tile last col at d = 2ll+5, 2ll+6 (tau=3 lanes)
        selc = np.zeros((BC, K, ND), np.float32)
        for i in range(BC):
            selc[i, :, 2 * llc[i] + 5] = 1.0
            selc[i, :, 2 * llc[i] + 6] = 1.0
        sel = selc.reshape(P, ND).astype(ml_dtypes.bfloat16)

        corr = np.repeat(corr_all[bs], K).astype(np.float32).reshape(P, 1)

        in_maps.append({
            "p_skew": np.ascontiguousarray(p_skew),
            "mu": np.ascontiguousarray(mu),
            "sel": np.ascontiguousarray(sel),
            "corr": corr,
            "sh": sh,
        })
    return in_maps


def kernel(y_pred, labels, input_len, label_len):
    y_pred = np.asarray(y_pred, np.float32)
    labels = np.asarray(labels, np.int32)
    input_len = np.asarray(input_len, np.int32)
    label_len = np.asarray(label_len, np.int32)
    assert np.all(input_len == T), "kernel assumes full-length inputs"

    from concourse.bass_utils import run_bass_kernel_spmd

    if "nc" not in _PROG_CACHE:
        _PROG_CACHE["nc"] = _build_program()
    nc = _PROG_CACHE["nc"]

    in_maps = _host_prep(y_pred, labels, label_len)
    res = run_bass_kernel_spmd(nc, in_maps, list(range(NCORES)))

    loss = np.zeros(B, np.float32)
    for c in range(NCORES):
        out = res.results[c]["loss_out"].reshape(P)
        loss[c * BC:(c + 1) * BC] = out[K - 1::K]
    return loss


# revision 16
# speedup vs baseline: 42.8264x; 42.8264x over previous
"""CTC loss (keras ctc_batch_cost semantics) on 8 Trainium2 NeuronCores.

Strategy (data parallel, 32 samples/core):
  Prob-domain CTC forward with per-sample/per-block prescaling.  The time
  recursion alpha_t = (c_{t-1} + alpha_{t-1}) * p_t is computed row-by-row
  (row = extended-label state s) with the DVE tensor_tensor_scan instruction
  (op0=add, op1=mult), one scan per (row, 128-step time block).

  Layout: partitions = (sample_local b in 0..31) x (time block tau in 0..3),
  free dim = t within block.  Work is ordered by skewed diagonals
  d = s + 2*tau so every diagonal has uniform blank/label parity and all
  cross-row references stay in-partition; the only cross-partition value is
  the scan's initial carry at block boundaries, produced by a tiny PE
  shift-matrix matmul accumulated into PSUM (scan reads `initial` from PSUM).

  Host-side prep (numpy): label-indexed gather of emissions into the skewed
  layout, bf16 cast, per-sample per-block power-of-two-ish prescale chosen
  from a coarse float64 estimate (pure preconditioning - correctness never
  depends on it; exact log-scale corrections are folded into the final loss
  constant per sample).
"""

import numpy as np
import ml_dtypes

B, T, C, L = 256, 512, 128, 64
S = 2 * L + 1          # 129 extended states
BLANK = C - 1
EPS = 1e-7
W = 128                # time-block width
K = 4                  # number of time blocks (T = K*W)
ND = S + 2 * (K - 1)   # diagonals: d = s + 2*tau in [0, 134]
NODD = (ND + 1) // 2   # odd diagonals (label rows)
NCORES = 8
BC = B // NCORES       # 32 samples per core
P = BC * K             # partitions used (64 for K=2)

_PROG_CACHE = {}


def _build_program():
    import concourse.bass as bass
    import concourse.bacc as bacc
    import concourse.mybir as mybir
    import concourse.tile as tile

    f32 = mybir.dt.float32
    bf16 = mybir.dt.bfloat16
    ADD = mybir.AluOpType.add
    MULT = mybir.AluOpType.mult
    CW = W + 1  # tile width: col 0 zero pad, cols 1..128 data

    nc = bacc.Bacc("TRN2", target_bir_lowering=False, debug=False)

    p_dram = nc.dram_tensor("p_skew", [P, ND * W], bf16, kind="ExternalInput")
    mu_dram = nc.dram_tensor("mu", [P, ND], f32, kind="ExternalInput")
    sel_dram = nc.dram_tensor("sel", [P, ND], bf16, kind="ExternalInput")
    corr_dram = nc.dram_tensor("corr", [P, 1], f32, kind="ExternalInput")
    sh_dram = nc.dram_tensor("sh", [P, P], bf16, kind="ExternalInput")
    out_dram = nc.dram_tensor("loss_out", [P, 1], f32, kind="ExternalOutput")

    with tile.TileContext(nc) as tc:
        with (
            tc.tile_pool(name="stat", bufs=1) as stat,
            tc.tile_pool(name="psum", bufs=8, space="PSUM") as psum,
        ):
            p_sb = stat.tile([P, ND * W], bf16, tag="p_sb")
            abuf = stat.tile([P, ND * CW], bf16, tag="abuf")
            cbuf = stat.tile([P, 4 * CW], bf16, tag="cbuf")
            wbuf = stat.tile([P, 4 * W], bf16, tag="wbuf")
            zw = stat.tile([P, CW], bf16, tag="zw")
            stg = stat.tile([P, ND], bf16, tag="stg")
            mu_sb = stat.tile([P, ND], f32, tag="mu_sb")
            sel_sb = stat.tile([P, ND], bf16, tag="sel_sb")
            corr_sb = stat.tile([P, 1], f32, tag="corr_sb")
            sh_sb = stat.tile([P, P], bf16, tag="sh_sb")
            rsel = stat.tile([P, ND], bf16, tag="rsel")
            r_col = stat.tile([P, 1], f32, tag="r_col")
            lnr = stat.tile([P, 1], f32, tag="lnr")
            eps_col = stat.tile([P, 1], f32, tag="eps_col")
            loss_sb = stat.tile([P, 1], f32, tag="loss_sb")

            chunks = [(0, 2), (2, 8), (10, 15)]
            i = 25
            while i < ND:
                chunks.append((i, min(18, ND - i)))
                i += 18
            first = True
            for i, wch in chunks:
                nc.sync.dma_start(
                    out=p_sb[:, i * W:(i + wch) * W],
                    in_=p_dram[:, i * W:(i + wch) * W],
                )
                if first:
                    # small tensors needed by the first diagonals; keep them
                    # ahead of the bulk p_skew load on the DMA queue
                    nc.sync.dma_start(out=sh_sb[:], in_=sh_dram[:])
                    nc.sync.dma_start(out=mu_sb[:], in_=mu_dram[:])
                    nc.sync.dma_start(out=sel_sb[:], in_=sel_dram[:])
                    nc.sync.dma_start(out=corr_sb[:], in_=corr_dram[:])
                    first = False

            a3 = abuf[:].rearrange("p (d c) -> p d c", c=CW)
            c3 = cbuf[:].rearrange("p (d c) -> p d c", c=CW)
            nc.vector.memset(a3[:, :, 0:1], 0.0)
            nc.vector.memset(c3[:, :, 0:1], 0.0)
            nc.vector.memset(zw[:], 0.0)
            nc.vector.memset(eps_col[:], 1e-35)

            def atile(d, lo, hi):
                return abuf[:, d * CW + lo: d * CW + hi]

            def ctile(oi, lo, hi):
                oi = oi % 4
                return cbuf[:, oi * CW + lo: oi * CW + hi]

            for d in range(ND):
                odd = (d % 2) == 1
                oi = (d - 1) // 2

                if d >= 2:
                    if odd:
                        in2 = ctile((d - 3) // 2, W, CW)
                    else:
                        in2 = atile(d - 3, W, CW) if d >= 3 else zw[:, 0:1]
                    nc.gpsimd.tensor_add(
                        stg[:, d:d + 1], atile(d - 2, W, CW), in2
                    )
                    pt = psum.tile([P, 1], f32, tag="init")
                    nc.tensor.matmul(
                        pt[:], sh_sb[:], stg[:, d:d + 1], start=True, stop=True
                    )
                    initial = pt[:, 0:1]
                else:
                    initial = 1.0

                if odd:
                    wi = ((d - 3) // 2) % 4
                    w_in = zw[:, 0:W] if d == 1 else wbuf[:, wi * W:wi * W + W]
                    nc.vector.tensor_add(
                        ctile(oi, 1, CW), atile(d - 1, 1, CW), w_in
                    )
                    data0 = ctile(oi, 0, W)
                else:
                    data0 = zw[:, 0:W] if d == 0 else atile(d - 1, 0, W)

                nc.vector.tensor_tensor_scan(
                    atile(d, 1, CW),
                    data0,
                    p_sb[:, d * W:(d + 1) * W],
                    initial,
                    op0=ADD,
                    op1=MULT,
                )

                if odd and d + 2 < ND:
                    wo = oi % 4
                    nc.scalar.activation(
                        wbuf[:, wo * W: wo * W + W],
                        atile(d, 1, CW),
                        mybir.ActivationFunctionType.Copy,
                        scale=mu_sb[:, d:d + 1],
                    )

            lastcols = a3[:, :, W]
            nc.vector.tensor_mul(rsel[:], lastcols, sel_sb[:])
            nc.vector.tensor_reduce(
                r_col[:], rsel[:], axis=mybir.AxisListType.X, op=ADD
            )
            nc.scalar.activation(
                lnr[:], r_col[:], mybir.ActivationFunctionType.Ln,
                bias=eps_col[:, 0:1],
            )
            nc.vector.tensor_scalar(
                loss_sb[:], lnr[:], -1.0, corr_sb[:, 0:1], op0=MULT, op1=ADD
            )
            nc.sync.dma_start(out=out_dram[:], in_=loss_sb[:])

    nc.compile()
    return nc


def _host_prep(y_pred, labels, label_len):
    """Build per-core device inputs. Returns list of in_maps + nothing else."""
    ll = label_len[:, 0].astype(np.int64)
    pe = y_pred.astype(np.float64) + EPS
    cls = np.full((B, S), BLANK, np.int64)
    cls[:, 1::2] = labels

    # gathered emissions [B, S, T], valid-masked (s <= 2*ll)
    em = np.take_along_axis(pe.transpose(0, 2, 1), cls[:, :, None], axis=1)
    valid = np.arange(S)[None, :] <= (2 * ll[:, None])
    em = em * valid[:, :, None]

    # skip mask per row
    mt = np.zeros((B, S), np.float32)
    j = np.arange(1, L)
    mt[:, 2 * j + 1] = (labels[:, j] != labels[:, j - 1]).astype(np.float32)

    # --- coarse scale estimation (float64, preconditioning only) --------
    SUB = 4
    mt64 = mt.astype(np.float64)
    a = np.zeros((B, S))
    a[:, 0] = em[:, 0, 0]
    a[:, 1] = em[:, 1, 0]
    logsc = np.zeros(B)
    blk_log = np.zeros((B, K))
    emc = em.reshape(B, S, T // SUB, SUB)
    for tc_ in range(1, T // SUB):
        pstep = emc[:, :, tc_, :].prod(axis=2) ** (1.0 / SUB)
        for _ in range(SUB):
            s1 = np.concatenate([np.zeros((B, 1)), a[:, :-1]], 1)
            s2 = np.concatenate([np.zeros((B, 2)), a[:, :-2]], 1) * mt64
            a = (a + s1 + s2) * pstep
        m = np.maximum(a.max(1), 1e-300)
        logsc += np.log2(m)
        a = a / m[:, None]
        blk_log[:, min(tc_ * SUB // W, K - 1)] = logsc
    deltas = np.diff(np.concatenate([np.zeros((B, 1)), blk_log], 1), axis=1)
    g = np.clip(-deltas / W, -30, 30)
    scale = (2.0 ** g).astype(np.float32)                     # [B, K]
    corr_all = (W * np.log(scale.astype(np.float64))).sum(1)  # [B] nats

    # scaled bf16 emissions
    emsc = np.zeros((B, S, T), np.float32)
    for tau in range(K):
        emsc[:, :, tau * W:(tau + 1) * W] = (
            em[:, :, tau * W:(tau + 1) * W] * scale[:, None, tau, None]
        )
    emsc = emsc.astype(ml_dtypes.bfloat16)

    # shift matrix (shared by all cores): out[p] = in[p-1] if p%4 != 0
    sh = np.zeros((P, P), np.float32)
    for p in range(P):
        if p % K != 0:
            sh[p - 1, p] = 1.0
    sh = sh.astype(ml_dtypes.bfloat16)

    in_maps = []
    for c in range(NCORES):
        bs = slice(c * BC, (c + 1) * BC)
        llc = ll[bs]
        # p_skew [BC, K, ND, W] -> [P, ND*W]
        psk = np.zeros((BC, K, ND, W), ml_dtypes.bfloat16)
        for tau in range(K):
            for d in range(2 * tau, 2 * tau + S):
                s = d - 2 * tau
                psk[:, tau, d, :] = emsc[bs, s, tau * W:(tau + 1) * W]
        p_skew = psk.reshape(BC * K, ND * W)

        # mu [P, ND]: mu[(b,tau), d] = mt[b, d-2tau+2] for odd d
        muc = np.zeros((BC, K, ND), np.float32)
        for tau in range(K):
            for d in range(1, ND, 2):
                s2 = d - 2 * tau + 2
                if 1 <= s2 < S and s2 % 2 == 1:
                    muc[:, tau, d] = mt[bs, s2]
        mu = muc.reshape(P, ND)

        # sel [P, ND]: pick alpha-tile last col at d = 2ll+5, 2ll+6 (tau=3 lanes)
        selc = np.zeros((BC, K, ND), np.float32)
        for i in range(BC):
            selc[i, :, 2 * llc[i] + 2 * (K - 1) - 1] = 1.0
            selc[i, :, 2 * llc[i] + 2 * (K - 1)] = 1.0
        sel = selc.reshape(P, ND).astype(ml_dtypes.bfloat16)

        corr = np.repeat(corr_all[bs], K).astype(np.float32).reshape(P, 1)

        in_maps.append({
            "p_skew": np.ascontiguousarray(p_skew),
            "mu": np.ascontiguousarray(mu),
            "sel": np.ascontiguousarray(sel),
            "corr": corr,
            "sh": sh,
        })
    return in_maps


def kernel(y_pred, labels, input_len, label_len):
    y_pred = np.asarray(y_pred, np.float32)
    labels = np.asarray(labels, np.int32)
    input_len = np.asarray(input_len, np.int32)
    label_len = np.asarray(label_len, np.int32)
    assert np.all(input_len == T), "kernel assumes full-length inputs"

    from concourse.bass_utils import run_bass_kernel_spmd

    if "nc" not in _PROG_CACHE:
        _PROG_CACHE["nc"] = _build_program()
    nc = _PROG_CACHE["nc"]

    in_maps = _host_prep(y_pred, labels, label_len)
    res = run_bass_kernel_spmd(nc, in_maps, list(range(NCORES)))

    loss = np.zeros(B, np.float32)
    for c in range(NCORES):
        out = res.results[c]["loss_out"].reshape(P)
        loss[c * BC:(c + 1) * BC] = out[K - 1::K]
    return loss
